# revision 5
# baseline (speedup 1.0000x reference)
"""Causal multi-head attention (B=2, S=2048, D=1024, H=16, Dh=64) on 8 trn2
NeuronCores.

Sharding: tensor-parallel over (batch x head-group). Core c handles batch
c//4 and heads [4*(c%4), 4*(c%4)+4). Each core computes its heads' Q/K/V
projections, causal softmax attention, and a partial output projection
(row-parallel Wo). Host sums the 4 partials per batch and adds bo.

Device-side layout ("scores-transposed"): the contraction dim always sits on
partitions so no transposes are ever needed:
  qT/kT: [head-dim on partitions, seq free]   (from W.T @ x.T)
  v:     [seq on partitions, head-dim free]   (from x @ Wv)
  scoresT[k, q] = kT-block.T @ qT-block       (k seq on partitions)
  softmax: exp on ACT (no max subtraction - scores are O(3) here); the row
           sums ride along the v matmul via an appended ones column; the
           1/sum broadcast is a K=1 matmul; normalization is one vector mul.
  out    = h_norm-blocks.T @ Wo-rows          (partial, summed on host)

All matmul operands are float32r (full PE rate at N>=256, ~1e-4 rel err).
Constraint honored throughout: matmul PSUM destinations must start at
partition 0, and a matmul with start=True zeroes its whole 2KB bank, so
even/odd head accumulation groups live in separate banks.
"""

import numpy as np

import concourse.bacc as bacc
import concourse.mybir as mybir
import concourse.tile as tile
from concourse import bass2jax

F32 = mybir.dt.float32
F32R = mybir.dt.float32r

B, S, D = 2, 2048, 1024
H_PER_CORE = 4          # heads per core
DH = 64                 # head dim
FW = H_PER_CORE * DH    # 256: per-core projection width
N_CORES = 8
QCHUNK = 512            # q columns processed per chunk
NQC = S // QCHUNK       # 4 chunks
KT = S // 128           # 16 k-tiles
# v_t per-s-tile layout, per head pair p at offset p*193:
#   [0:64]=v_even  [64:65]=1 (even sums row 64)  [65:66]=1 (odd sums row 0)
#   [66:129]=unused  [129:193]=v_odd (odd out rows 64:128)
VSEG = 193
VBLK = 2 * VSEG         # 386 per s-tile


def build_nc(reps: int = 1):
    nc = bacc.Bacc("TRN2", target_bir_lowering=False, debug=False)

    xT = nc.dram_tensor("xT", [D, S], F32R, kind="ExternalInput")
    wq = nc.dram_tensor("wq", [D, FW], F32R, kind="ExternalInput")
    wk = nc.dram_tensor("wk", [D, FW], F32R, kind="ExternalInput")
    wv = nc.dram_tensor("wv", [D, FW], F32R, kind="ExternalInput")
    wo = nc.dram_tensor("wo", [FW, D], F32R, kind="ExternalInput")
    bq = nc.dram_tensor("bq", [FW, 1], F32, kind="ExternalInput")
    bk = nc.dram_tensor("bk", [FW, 1], F32, kind="ExternalInput")
    bvb = nc.dram_tensor("bvb", [128, FW], F32, kind="ExternalInput")
    ones = nc.dram_tensor("ones", [128, 128], F32R, kind="ExternalInput")
    maskg = nc.dram_tensor("maskg", [128, 2048], F32R, kind="ExternalInput")
    out = nc.dram_tensor("out", [S, D], F32, kind="ExternalOutput")

    with tile.TileContext(nc) as tc, nc.allow_low_precision(
            reason="float32r matmul operands carry reduced mantissas by design"):
        for _ in range(reps):
            _emit_body(nc, tc, xT, wq, wk, wv, wo, bq, bk, bvb, ones, maskg,
                       out)
    nc.compile()
    return nc


def _emit_body(nc, tc, xT, wq, wk, wv, wo, bq, bk, bvb, ones, maskg, out):
    with tc.tile_pool(name="wpool", bufs=1) as wpool, \
         tc.tile_pool(name="qkv", bufs=1) as qkv:
        # --- weights + aux ---
        wq_t = wpool.tile([128, 8 * FW], F32R)   # [d-in-tile, (d-tile, f)]
        wk_t = wpool.tile([128, 8 * FW], F32R)
        wv_t = wpool.tile([128, 8 * FW], F32R)
        wo_t = wpool.tile([128, 2 * D], F32R)    # [fw-in-tile, (fw-tile, n)]
        bq_t = wpool.tile([128, 2], F32)
        bk_t = wpool.tile([128, 2], F32)
        bvb_t = wpool.tile([128, FW], F32)
        ones_t = wpool.tile([128, 128], F32R)
        maskg_t = wpool.tile([128, 2048], F32R)
        for d in range(8):
            nc.sync.dma_start(out=wq_t[:, d * FW:(d + 1) * FW],
                              in_=wq[d * 128:(d + 1) * 128, :])
            nc.sync.dma_start(out=wk_t[:, d * FW:(d + 1) * FW],
                              in_=wk[d * 128:(d + 1) * 128, :])
            nc.sync.dma_start(out=wv_t[:, d * FW:(d + 1) * FW],
                              in_=wv[d * 128:(d + 1) * 128, :])
        for t in range(2):
            nc.sync.dma_start(out=wo_t[:, t * D:(t + 1) * D],
                              in_=wo[t * 128:(t + 1) * 128, :])
            nc.sync.dma_start(out=bq_t[:, t:t + 1],
                              in_=bq[t * 128:(t + 1) * 128, :])
            nc.sync.dma_start(out=bk_t[:, t:t + 1],
                              in_=bk[t * 128:(t + 1) * 128, :])
        nc.sync.dma_start(out=bvb_t[:], in_=bvb[:])
        nc.sync.dma_start(out=ones_t[:], in_=ones[:])
        nc.sync.dma_start(out=maskg_t[:], in_=maskg[:])

        # --- projections ---
        qT = [qkv.tile([128, S], F32R, name=f"qT{p}") for p in range(2)]
        kT = [qkv.tile([128, S], F32R, name=f"kT{p}") for p in range(2)]
        v_t = qkv.tile([128, KT * VBLK], F32R)
        # ones columns of v_t (positions 64,65 within each 193-block)
        nc.vector.tensor_copy(
            v_t[:].rearrange("x (s p b) -> x s p b", s=KT, p=2)[:, :, :, 64:66],
            ones_t[:, 0:64].rearrange("x (s p b) -> x s p b", s=KT, p=2))

        with tc.tile_pool(name="xtp", bufs=1) as xtp, \
             tc.tile_pool(name="ppp", bufs=2, space="PSUM") as ppp:
            xt = []
            for d in range(8):
                x_d = xtp.tile([128, S], F32R, name=f"xt{d}")
                nc.sync.dma_start(out=x_d[:], in_=xT[d * 128:(d + 1) * 128, :])
                xt.append(x_d)
            # qT / kT: out[f, s] accumulated over d
            for dst, w_t, b_t in ((qT, wq_t, bq_t), (kT, wk_t, bk_t)):
                for p in range(2):
                    for sc in range(4):
                        pt = ppp.tile([128, 512], F32, name="pt")
                        for d in range(8):
                            nc.tensor.matmul(
                                pt[:],
                                w_t[:, d * FW + p * 128: d * FW + (p + 1) * 128],
                                xt[d][:, sc * 512:(sc + 1) * 512],
                                start=(d == 0), stop=(d == 7),
                            )
                        nc.vector.tensor_scalar_add(
                            dst[p][:, sc * 512:(sc + 1) * 512], pt[:],
                            b_t[:, p:p + 1])
            # v: out[s, f] per s-tile, scattered into the v_t segment layout
            for st in range(KT):
                pt = ppp.tile([128, FW], F32, name="pt")
                for d in range(8):
                    nc.tensor.matmul(
                        pt[:],
                        xt[d][:, st * 128:(st + 1) * 128],
                        wv_t[:, d * FW:(d + 1) * FW],
                        start=(d == 0), stop=(d == 7),
                    )
                seg = v_t[:, st * VBLK:(st + 1) * VBLK].rearrange(
                    "x (p b) -> x p b", p=2)
                pt4 = pt[:].rearrange("x (h c) -> x h c", h=4)
                bv4 = bvb_t[:].rearrange("x (h c) -> x h c", h=4)
                nc.vector.tensor_add(seg[:, :, 0:64], pt4[:, 0:4:2, :],
                                     bv4[:, 0:4:2, :])
                nc.vector.tensor_add(seg[:, :, 129:193], pt4[:, 1:4:2, :],
                                     bv4[:, 1:4:2, :])

        # --- attention + output projection ---
        # PSUM budget (8 banks): scores 4 + h 2 + aux(shared slot) 2 = 8.
        with tc.tile_pool(name="spp", bufs=1, space="PSUM") as spp, \
             tc.tile_pool(name="hpp", bufs=1, space="PSUM") as hpp, \
             tc.tile_pool(name="aux_pp", bufs=1, space="PSUM") as aux_pp, \
             tc.tile_pool(name="expw", bufs=3) as expw_pool, \
             tc.tile_pool(name="sm", bufs=2) as sm_pool, \
             tc.tile_pool(name="hn", bufs=4) as hn_pool, \
             tc.tile_pool(name="op", bufs=2) as op_pool:
            for J in range(NQC):
                n_ki = 4 * J + 4
                qs = slice(J * 512, (J + 1) * 512)
                hn_t = [None, None]
                for p in range(2):
                    # h_ps bank 0: even head rows [0:64]=h, [64:65]=sums
                    # h_ps bank 1: odd head  rows [0:1]=sums, [64:128]=h
                    h_ps = hpp.tile([128, 1024], F32, name="h_ps")
                    vbase = p * VSEG
                    for w in range((n_ki + 1) // 2):
                        kis = [2 * w, 2 * w + 1]
                        sc_ps = spp.tile([128, 2048], F32, name="sc_ps")
                        for i, ki in enumerate(kis):
                            # scores^T: row-tiled head pair (K=64 each)
                            nc.tensor.matmul(
                                sc_ps[:, i * 512:(i + 1) * 512],
                                kT[p][0:64, ki * 128:(ki + 1) * 128],
                                qT[p][0:64, qs],
                                start=True, stop=True, tile_position=(0, 0),
                            )
                            nc.tensor.matmul(
                                sc_ps[:, 1024 + i * 512: 1024 + (i + 1) * 512],
                                kT[p][64:128, ki * 128:(ki + 1) * 128],
                                qT[p][64:128, qs],
                                start=True, stop=True, tile_position=(64, 0),
                            )
                        ew = expw_pool.tile([128, 2048], F32R, name="ew")
                        nc.scalar.activation(
                            ew[:], sc_ps[:], mybir.ActivationFunctionType.Exp)
                        if 2 * w >= 4 * J:  # diagonal wave
                            moff = (2 * w - 4 * J) * 512
                            nc.vector.tensor_mul(
                                ew[:, 0:1024], ew[:, 0:1024],
                                maskg_t[:, moff:moff + 1024])
                            nc.vector.tensor_mul(
                                ew[:, 1024:2048], ew[:, 1024:2048],
                                maskg_t[:, moff:moff + 1024])
                        for i, ki in enumerate(kis):
                            # h + sums in one matmul per head (ones col in v_t)
                            nc.tensor.matmul(
                                h_ps[0:65, 0:512],
                                v_t[:, ki * VBLK + vbase:
                                    ki * VBLK + vbase + 65],
                                ew[:, i * 512:(i + 1) * 512],
                                start=(ki == 0), stop=(ki == n_ki - 1),
                            )
                            nc.tensor.matmul(
                                h_ps[0:128, 512:1024],
                                v_t[:, ki * VBLK + vbase + 65:
                                    ki * VBLK + vbase + VSEG],
                                ew[:, 1024 + i * 512: 1024 + (i + 1) * 512],
                                start=(ki == 0), stop=(ki == n_ki - 1),
                            )
                    # normalization: 1/sums, broadcast via K=1 matmuls
                    rec_t = sm_pool.tile([128, 1024], F32R, name="rec_t")
                    nc.vector.reciprocal(rec_t[64:65, 0:512],
                                         h_ps[64:65, 0:512])
                    nc.vector.reciprocal(rec_t[0:1, 512:1024],
                                         h_ps[0:1, 512:1024])
                    bc_ps = aux_pp.tile([128, 1024], F32, name="bc_ps",
                                        tag="aux")
                    nc.tensor.matmul(bc_ps[:, 0:512],
                                     ones_t[64:65, :],
                                     rec_t[64:65, 0:512],
                                     start=True, stop=True,
                                     tile_position=(64, 0))
                    nc.tensor.matmul(bc_ps[:, 512:1024],
                                     ones_t[0:1, :],
                                     rec_t[0:1, 512:1024],
                                     start=True, stop=True,
                                     tile_position=(0, 0))
                    bc_t = sm_pool.tile([128, 1024], F32, name="bc_t")
                    nc.vector.tensor_copy(bc_t[:], bc_ps[:])
                    hn = hn_pool.tile([128, 512], F32R, name="hn")
                    nc.vector.tensor_mul(hn[0:64, :], h_ps[0:64, 0:512],
                                         bc_t[0:64, 0:512])
                    nc.vector.tensor_mul(hn[64:128, :],
                                         h_ps[64:128, 512:1024],
                                         bc_t[64:128, 512:1024])
                    hn_t[p] = hn
                # output projection for this chunk
                for m in range(4):
                    o_t = op_pool.tile([128, D], F32, name="o_t")
                    for n in range(2):
                        o_ps = aux_pp.tile([128, 512], F32, name="o_ps",
                                           tag="aux")
                        for p in range(2):
                            nc.tensor.matmul(
                                o_ps[:],
                                hn_t[p][:, m * 128:(m + 1) * 128],
                                wo_t[:, p * D + n * 512: p * D + (n + 1) * 512],
                                start=(p == 0), stop=(p == 1),
                            )
                        nc.vector.tensor_copy(o_t[:, n * 512:(n + 1) * 512],
                                              o_ps[:])
                    nc.sync.dma_start(
                        out=out[J * 512 + m * 128: J * 512 + (m + 1) * 128, :],
                        in_=o_t[:])


class _Runner:
    """Jitted SPMD executor over the 8 axon-tunneled NeuronCores."""

    def __init__(self, nc, n_cores=N_CORES):
        import jax
        from jax.sharding import Mesh, PartitionSpec, NamedSharding
        from jax.experimental.shard_map import shard_map

        self.jax = jax
        bass2jax.install_neuronx_cc_hook()
        partition_name = (
            nc.partition_id_tensor.name if nc.partition_id_tensor else None
        )
        in_names, out_names, out_avals, zero_outs = [], [], [], []
        for alloc in nc.m.functions[0].allocations:
            if not isinstance(alloc, mybir.MemoryLocationSet):
                continue
            name = alloc.memorylocations[0].name
            if alloc.kind == "ExternalInput":
                if name != partition_name:
                    in_names.append(name)
            elif alloc.kind == "ExternalOutput":
                out_names.append(name)
                shape = tuple(alloc.tensor_shape)
                dtype = mybir.dt.np(alloc.dtype)
                out_avals.append(jax.core.ShapedArray(shape, dtype))
                zero_outs.append(np.zeros(shape, dtype))
        self.in_names = in_names
        self.out_names = out_names
        self.out_avals = out_avals
        self.zero_outs = zero_outs
        self.n_cores = n_cores
        all_in = list(in_names) + list(out_names)
        if partition_name is not None:
            all_in.append(partition_name)

        def _body(*args):
            operands = list(args)
            if partition_name is not None:
                operands.append(bass2jax.partition_id_tensor())
            outs = bass2jax._bass_exec_p.bind(
                *operands,
                out_avals=tuple(out_avals),
                in_names=tuple(all_in),
                out_names=tuple(out_names),
                lowering_input_output_aliases=(),
                sim_require_finite=True,
                sim_require_nnan=True,
                nc=nc,
            )
            return tuple(outs)

        devices = jax.devices()[:n_cores]
        assert len(devices) == n_cores
        self.mesh = Mesh(np.asarray(devices), ("core",))
        n_in = len(in_names) + len(out_names)
        self.fn = jax.jit(
            shard_map(
                _body, mesh=self.mesh,
                in_specs=(PartitionSpec("core"),) * n_in,
                out_specs=(PartitionSpec("core"),) * len(out_names),
                check_rep=False,
            ),
            keep_unused=True,
        )
        self.sharding = NamedSharding(self.mesh, PartitionSpec("core"))

    def put_inputs(self, in_maps):
        concat_in = [
            np.concatenate(
                [np.asarray(in_maps[c][n]) for c in range(self.n_cores)], axis=0
            )
            for n in self.in_names
        ]
        concat_zeros = [
            np.zeros((self.n_cores * z.shape[0], *z.shape[1:]), z.dtype)
            for z in self.zero_outs
        ]
        args = [
            self.jax.device_put(a, self.sharding)
            for a in concat_in + concat_zeros
        ]
        self.jax.block_until_ready(args)
        return args

    def run(self, args):
        out = self.fn(*args)
        self.jax.block_until_ready(out)
        return out

    def split_outputs(self, out_arrs):
        return [
            {
                n: np.asarray(out_arrs[i]).reshape(
                    self.n_cores, *self.out_avals[i].shape)[c]
                for i, n in enumerate(self.out_names)
            }
            for c in range(self.n_cores)
        ]


def make_core_inputs(x, Wq, bq, Wk, bk, Wv, bv, Wo):
    """Host-side slicing for the 8 cores. Wq/bq are pre-scaled by 1/sqrt(Dh)."""
    ones = np.ones((128, 128), np.float32)
    k_idx = np.arange(128)[:, None]
    q_idx = np.arange(512)[None, :]
    maskg = np.concatenate(
        [(k_idx <= q_idx - 128 * m).astype(np.float32) for m in range(4)],
        axis=1)
    in_maps = []
    xTb = [np.ascontiguousarray(x[b].T) for b in range(B)]
    for c in range(N_CORES):
        b, g = c // 4, c % 4
        fs = slice(g * FW, (g + 1) * FW)
        in_maps.append({
            "xT": xTb[b],
            "wq": np.ascontiguousarray(Wq[:, fs]),
            "wk": np.ascontiguousarray(Wk[:, fs]),
            "wv": np.ascontiguousarray(Wv[:, fs]),
            "wo": np.ascontiguousarray(Wo[fs, :]),
            "bq": np.ascontiguousarray(bq[fs]).reshape(FW, 1),
            "bk": np.ascontiguousarray(bk[fs]).reshape(FW, 1),
            "bvb": np.broadcast_to(bv[fs], (128, FW)).copy(),
            "ones": ones,
            "maskg": maskg,
        })
    return in_maps


_CACHE = {}


def get_runner(reps: int = 1):
    key = reps
    if key not in _CACHE:
        _CACHE[key] = _Runner(build_nc(reps))
    return _CACHE[key]


def kernel(x, Wq, bq, Wk, bk, Wv, bv, Wo, bo):
    x = np.asarray(x, np.float32)
    scale = np.float32(1.0 / np.sqrt(DH))
    in_maps = make_core_inputs(
        x,
        np.asarray(Wq, np.float32) * scale, np.asarray(bq, np.float32) * scale,
        np.asarray(Wk, np.float32), np.asarray(bk, np.float32),
        np.asarray(Wv, np.float32), np.asarray(bv, np.float32),
        np.asarray(Wo, np.float32))
    r = get_runner()
    args = r.put_inputs(in_maps)
    outs = r.split_outputs(r.run(args))
    result = np.zeros((B, S, D), np.float32)
    for c in range(N_CORES):
        result[c // 4] += outs[c]["out"]
    result += np.asarray(bo, np.float32)
    return result
